# revision 2
# baseline (speedup 1.0000x reference)
"""Trainium2 Bass kernel for nn_CRF (gnn_message_passing).

Reference computation (per batch b of 256):
    sim   = (F F^T) / (|f_n||f_m|)        F = feats[b]  [N=256, E=512]
    P     = sim * W_sym                   W_sym = (W + W^T)/2  [N, N]
    lg_0  = logits[b]                     [N, 1]
    10x:  lg = logits + P @ (2*sigmoid(lg) - 1)     (2*sig(x)-1 == tanh(x/2))

Strategy: pure data parallel, 32 batches per core on 8 NeuronCores.
Per core we build A = (F F^T) * W_sym (symmetric, unnormalized) in SBUF as
bf16, compute r = 1/|f| once, and run the 10 CRF iterations fully on-chip:
    vt = r * tanh(0.5 * lg)    e~ = A @ vt     lg = logits + r * e~
All iteration-state tensors live in a dense [128, 64] layout:
partition p = n mod 128, column = (n div 128)*32 + batch.
"""

import sys

sys.path.insert(0, "/opt/trn_rl_repo")

from contextlib import ExitStack

import ml_dtypes
import numpy as np

import concourse.bacc as bacc
import concourse.mybir as mybir
import concourse.tile as tile
from concourse.bass_utils import run_bass_kernel_spmd

B, N, E, ITER = 256, 256, 512, 10
NCORES = 8
BP = B // NCORES  # 32 batches per core
P = 128  # partitions
NH = N // P  # 2 halves of the node dim
EC = E // P  # 4 chunks of the embedding dim
NG = 8  # transpose-DMA batch groups
GB = BP // NG  # 4 batches per group
COLS = NH * BP  # 64 = columns of the [128, 64] iteration-state layout

F32 = mybir.dt.float32
BF16 = mybir.dt.bfloat16
AF = mybir.ActivationFunctionType

_CACHE: dict = {}


def _build_nc():
    nc = bacc.Bacc(
        "TRN2",
        target_bir_lowering=False,
        debug=False,
        enable_asserts=False,
        num_devices=NCORES,
    )

    fbf = nc.dram_tensor("fbf", [BP * N, E], BF16, kind="ExternalInput").ap()
    logT = nc.dram_tensor("logT", [P, COLS], F32, kind="ExternalInput").ap()
    wsym = nc.dram_tensor("wsym", [N, N], F32, kind="ExternalInput").ap()
    outT = nc.dram_tensor("outT", [P, COLS], F32, kind="ExternalOutput").ap()

    with tile.TileContext(nc) as tc, ExitStack() as ctx:
        cpool = ctx.enter_context(tc.tile_pool(name="cpool", bufs=1))
        ftp_pool = ctx.enter_context(tc.tile_pool(name="ftp", bufs=1))
        fnat_pool = ctx.enter_context(tc.tile_pool(name="fnat", bufs=3))
        scr_pool = ctx.enter_context(tc.tile_pool(name="scr", bufs=3))
        a_pool = ctx.enter_context(tc.tile_pool(name="apool", bufs=1))
        it_pool = ctx.enter_context(tc.tile_pool(name="itpool", bufs=2))
        psumD = ctx.enter_context(tc.tile_pool(name="psumD", bufs=2, space="PSUM"))
        psumE = ctx.enter_context(tc.tile_pool(name="psumE", bufs=2, space="PSUM"))

        # ---- constants ----
        w_sb = [cpool.tile([P, N], F32, tag=f"wsym{h}", name=f"wsym{h}") for h in range(NH)]
        for h in range(NH):
            nc.sync.dma_start(w_sb[h][:], wsym[h * P : (h + 1) * P, :])
        logT_sb = cpool.tile([P, COLS], F32, tag="logT", name="logT_sb")
        nc.sync.dma_start(logT_sb[:], logT)

        # ---- F^T tiles (DMA transpose): ftp[c][g][p, j*N + n] =
        #      feats[b = g*GB + j, n, c*128 + p] ----
        ftp = [
            [ftp_pool.tile([P, GB * N], BF16, tag=f"ftp{c}_{g}", name=f"ftp{c}_{g}") for g in range(NG)]
            for c in range(EC)
        ]
        for g in range(NG):
            for c in range(EC):
                nc.sync.dma_start_transpose(
                    ftp[c][g][:],
                    fbf[g * GB * N : (g + 1) * GB * N, c * P : (c + 1) * P],
                )

        # ---- norms: n2[p, h*BP+b] = sum_e feats[b, h*128+p, e]^2 ----
        n2 = cpool.tile([P, COLS], F32, tag="n2", name="n2")
        nblk = NH * GB  # 8 row-blocks of 128 per group
        fview = fbf.rearrange("(blk p) e -> p blk e", p=P)
        for g in range(NG):
            fn = fnat_pool.tile([P, nblk, E], BF16, name="fn")
            nc.sync.dma_start(fn[:], fview[:, g * nblk : (g + 1) * nblk, :])
            for blk in range(nblk):
                b, h = divmod(g * nblk + blk, NH)
                col = h * BP + b
                scr = scr_pool.tile([P, E], BF16, name="scr")
                nc.scalar.activation(
                    scr[:],
                    fn[:, blk, :],
                    AF.Square,
                    accum_out=n2[:, col : col + 1],
                )
        ns = cpool.tile([P, COLS], F32, tag="ns", name="ns")
        nc.scalar.activation(ns[:], n2[:], AF.Sqrt)
        r = cpool.tile([P, COLS], F32, tag="r", name="r")
        nc.vector.reciprocal(r[:], ns[:])

        # ---- pairwise potentials: A[b][h][p, m] = dot(f_{h*128+p}, f_m) * W_sym ----
        a_tiles = [
            [a_pool.tile([P, N], BF16, tag=f"A{b}_{h}", name=f"A{b}_{h}") for h in range(NH)]
            for b in range(BP)
        ]
        for b in range(BP):
            g, j = divmod(b, GB)
            pD = psumD.tile([P, NH * N], F32, name="pD")
            for h in range(NH):
                for c in range(EC):
                    nc.tensor.matmul(
                        pD[:, h * N : (h + 1) * N],
                        ftp[c][g][:, j * N + h * P : j * N + (h + 1) * P],
                        ftp[c][g][:, j * N : (j + 1) * N],
                        start=(c == 0),
                        stop=(c == EC - 1),
                    )
            for h in range(NH):
                nc.vector.tensor_mul(
                    a_tiles[b][h][:], pD[:, h * N : (h + 1) * N], w_sb[h][:]
                )

        # ---- CRF iterations ----
        v0 = it_pool.tile([P, COLS], F32, tag="v", name="v")
        nc.scalar.activation(v0[:], logT_sb[:], AF.Tanh, scale=0.5)
        vt = it_pool.tile([P, COLS], BF16, tag="vt", name="vt")
        nc.vector.tensor_mul(vt[:], v0[:], r[:])

        out_sb = cpool.tile([P, COLS], F32, tag="out_sb", name="out_sb")
        for t in range(ITER):
            pE = psumE.tile([P, COLS], F32, name="pE")
            for b in range(BP):
                for h in range(NH):
                    col = h * BP + b
                    for hp in range(NH):
                        nc.tensor.matmul(
                            pE[:, col : col + 1],
                            a_tiles[b][hp][:, h * P : (h + 1) * P],
                            vt[:, hp * BP + b : hp * BP + b + 1],
                            start=(hp == 0),
                            stop=(hp == NH - 1),
                        )
            er = it_pool.tile([P, COLS], F32, tag="er", name="er")
            nc.vector.tensor_mul(er[:], pE[:], r[:])
            if t < ITER - 1:
                lg = it_pool.tile([P, COLS], F32, tag="lg", name="lg")
                nc.vector.tensor_add(lg[:], er[:], logT_sb[:])
                vnew = it_pool.tile([P, COLS], F32, tag="v", name="v")
                nc.scalar.activation(vnew[:], lg[:], AF.Tanh, scale=0.5)
                vt = it_pool.tile([P, COLS], BF16, tag="vt", name="vt")
                nc.vector.tensor_mul(vt[:], vnew[:], r[:])
            else:
                nc.vector.tensor_add(out_sb[:], er[:], logT_sb[:])
        nc.sync.dma_start(outT, out_sb[:])

    nc.compile()
    return nc


def _get_nc():
    if "nc" not in _CACHE:
        _CACHE["nc"] = _build_nc()
    return _CACHE["nc"]


def _make_in_maps(feats, logits, W):
    wsym = ((W[0] + W[0].T) * 0.5).astype(np.float32)
    in_maps = []
    for i in range(NCORES):
        fs = np.ascontiguousarray(feats[i * BP : (i + 1) * BP]).reshape(BP * N, E)
        fs = fs.astype(ml_dtypes.bfloat16)
        lg = logits[i * BP : (i + 1) * BP, :, 0].astype(np.float32)
        lgT = np.ascontiguousarray(
            lg.reshape(BP, NH, P).transpose(2, 1, 0).reshape(P, COLS)
        )
        in_maps.append({"fbf": fs, "logT": lgT, "wsym": wsym})
    return in_maps


def _unshard(results):
    outs = []
    for i in range(NCORES):
        oT = np.asarray(results[i]["outT"], dtype=np.float32)  # [P, COLS]
        oc = oT.reshape(P, NH, BP).transpose(2, 1, 0).reshape(BP, N)
        outs.append(oc)
    return np.concatenate(outs, axis=0).reshape(B, N, 1).astype(np.float32)


def run(feats, logits, W, trace=False, **kwargs):
    nc = _get_nc()
    in_maps = _make_in_maps(np.asarray(feats), np.asarray(logits), np.asarray(W))
    res = run_bass_kernel_spmd(
        nc, in_maps, core_ids=list(range(NCORES)), trace=trace, **kwargs
    )
    return _unshard(res.results), res


def kernel(feats, logits, W):
    out, _ = run(feats, logits, W)
    return out


# revision 4
# speedup vs baseline: 1.7142x; 1.7142x over previous
"""Trainium2 Bass kernel for nn_CRF (gnn_message_passing).

Reference computation (per batch b of 256):
    sim   = (F F^T) / (|f_n||f_m|)        F = feats[b]  [N=256, E=512]
    P     = sim * W_sym                   W_sym = (W + W^T)/2  [N, N]
    lg_0  = logits[b]                     [N, 1]
    10x:  lg = logits + P @ (2*sigmoid(lg) - 1)     (2*sig(x)-1 == tanh(x/2))

Strategy: pure data parallel, 32 batches per core on 8 NeuronCores.
Per core we build A = (F F^T) * W_sym (symmetric, unnormalized) in SBUF as
bf16, compute r = 1/|f| once (PE ones-matvec over squared features), and run
the 10 CRF iterations fully on-chip:
    vt = r * tanh(0.5 * lg)    e~ = A @ vt     lg = logits + r * e~
Iteration state lives in a dense [128, 64] layout:
partition p = n mod 128, column = 32*(b div 16) + 16*(n div 128) + (b mod 16).
The batch halves (G = b div 16) are pipelined so VectorE/ScalarE work of one
half overlaps TensorE matvecs of the other.

feats are uploaded twice-transformed on host: bf16 cast + [E, BP*N] transpose
(pure layout prep), so the device only issues large contiguous DMAs.
"""

import sys

sys.path.insert(0, "/opt/trn_rl_repo")

from contextlib import ExitStack

import ml_dtypes
import numpy as np

import concourse.bacc as bacc
import concourse.mybir as mybir
import concourse.tile as tile
from concourse.bass_utils import run_bass_kernel_spmd

B, N, E, ITER = 256, 256, 512, 10
NCORES = 8
BP = B // NCORES  # 32 batches per core
P = 128  # partitions
NH = N // P  # 2 halves of the node dim
EC = E // P  # 4 chunks of the embedding dim
NG = 8  # DMA batch groups
GB = BP // NG  # 4 batches per group
COLS = NH * BP  # 64 columns of iteration-state layout
PG = 2  # pipeline groups over batches
PGB = BP // PG  # 16 batches per pipeline group
GCOLS = COLS // PG  # 32 columns per pipeline group

F32 = mybir.dt.float32
BF16 = mybir.dt.bfloat16
AF = mybir.ActivationFunctionType

_CACHE: dict = {}


def _col(b, h):
    g, lb = divmod(b, PGB)
    return GCOLS * g + PGB * h + lb


def _build_nc():
    nc = bacc.Bacc(
        "TRN2",
        target_bir_lowering=False,
        debug=False,
        enable_asserts=False,
        num_devices=NCORES,
    )

    ftT = nc.dram_tensor("ftT", [E, BP * N], BF16, kind="ExternalInput").ap()
    logT = nc.dram_tensor("logT", [P, COLS], F32, kind="ExternalInput").ap()
    wsym = nc.dram_tensor("wsym", [N, N], F32, kind="ExternalInput").ap()
    outT = nc.dram_tensor("outT", [P, COLS], F32, kind="ExternalOutput").ap()

    with tile.TileContext(nc) as tc, ExitStack() as ctx:
        cpool = ctx.enter_context(tc.tile_pool(name="cpool", bufs=1))
        ftp_pool = ctx.enter_context(tc.tile_pool(name="ftp", bufs=1))
        sq_pool = ctx.enter_context(tc.tile_pool(name="sq", bufs=10))
        a_pool = ctx.enter_context(tc.tile_pool(name="apool", bufs=1))
        it_pool = ctx.enter_context(tc.tile_pool(name="itpool", bufs=2))
        psumD = ctx.enter_context(tc.tile_pool(name="psumD", bufs=3, space="PSUM"))
        psumN = ctx.enter_context(tc.tile_pool(name="psumN", bufs=1, space="PSUM"))
        psumE = ctx.enter_context(tc.tile_pool(name="psumE", bufs=2, space="PSUM"))

        # ---- constants ----
        w_sb = [cpool.tile([P, N], F32, tag=f"wsym{h}", name=f"wsym{h}") for h in range(NH)]
        for h in range(NH):
            nc.sync.dma_start(w_sb[h][:], wsym[h * P : (h + 1) * P, :])
        logT_sb = cpool.tile([P, COLS], F32, tag="logT", name="logT_sb")
        nc.sync.dma_start(logT_sb[:], logT)
        ones = cpool.tile([P, 1], BF16, tag="ones", name="ones")
        nc.vector.memset(ones[:], 1.0)

        n2 = psumN.tile([P, COLS], F32, name="n2")

        # ---- per-group: load F^T, square, pairwise matmuls, norm matvecs ----
        # ftp[c][g][p, j*N + n] = feats[b = g*GB + j, n, c*128 + p]
        ftp = [
            [
                ftp_pool.tile([P, GB * N], BF16, tag=f"ftp{c}_{g}", name=f"ftp{c}_{g}")
                for g in range(NG)
            ]
            for c in range(EC)
        ]
        a_tiles = [
            [a_pool.tile([P, N], BF16, tag=f"A{b}_{h}", name=f"A{b}_{h}") for h in range(NH)]
            for b in range(BP)
        ]

        for g in range(NG):
            sq = []
            for c in range(EC):
                nc.sync.dma_start(
                    ftp[c][g][:],
                    ftT[c * P : (c + 1) * P, g * GB * N : (g + 1) * GB * N],
                )
                s = sq_pool.tile([P, GB * N], BF16, name="sq")
                # split squares between ScalarE and VectorE
                if (g * EC + c) % 2 == 0:
                    nc.scalar.activation(s[:], ftp[c][g][:], AF.Square)
                else:
                    nc.vector.tensor_mul(s[:], ftp[c][g][:], ftp[c][g][:])
                sq.append(s)

            for j in range(GB):
                b = g * GB + j
                # pairwise dots -> psum_D
                pD = psumD.tile([P, NH * N], F32, name="pD")
                for h in range(NH):
                    for c in range(EC):
                        nc.tensor.matmul(
                            pD[:, h * N : (h + 1) * N],
                            ftp[c][g][:, j * N + h * P : j * N + (h + 1) * P],
                            ftp[c][g][:, j * N : (j + 1) * N],
                            start=(c == 0),
                            stop=(c == EC - 1),
                        )
                # norm matvecs: n2[:, col(b,h)] = sum_e f^2
                for h in range(NH):
                    col = _col(b, h)
                    for c in range(EC):
                        nc.tensor.matmul(
                            n2[:, col : col + 1],
                            sq[c][:, j * N + h * P : j * N + (h + 1) * P],
                            ones[:],
                            start=(c == 0),
                            stop=(c == EC - 1),
                        )
                # A = D * W_sym
                for h in range(NH):
                    nc.vector.tensor_mul(
                        a_tiles[b][h][:], pD[:, h * N : (h + 1) * N], w_sb[h][:]
                    )

        # ---- r = 1/sqrt(n2) ----
        ns = cpool.tile([P, COLS], F32, tag="ns", name="ns")
        nc.scalar.activation(ns[:], n2[:], AF.Sqrt)
        r = cpool.tile([P, COLS], F32, tag="r", name="r")
        nc.vector.reciprocal(r[:], ns[:])

        # ---- CRF iterations, pipelined over PG batch groups ----
        def rsl(t_, g_):
            return t_[:, GCOLS * g_ : GCOLS * (g_ + 1)]

        vts = []
        for g in range(PG):
            v0 = it_pool.tile([P, GCOLS], F32, tag=f"v{g}", name=f"v{g}")
            nc.scalar.activation(v0[:], rsl(logT_sb, g), AF.Tanh, scale=0.5)
            vt = it_pool.tile([P, GCOLS], BF16, tag=f"vt{g}", name=f"vt{g}")
            nc.vector.tensor_mul(vt[:], v0[:], rsl(r, g))
            vts.append(vt)

        out_sb = cpool.tile([P, COLS], F32, tag="out_sb", name="out_sb")
        for t in range(ITER):
            pEs = []
            for g in range(PG):
                pE = psumE.tile([P, GCOLS], F32, name=f"pE{g}", tag=f"pE{g}")
                for lb in range(PGB):
                    b = g * PGB + lb
                    for h in range(NH):
                        for hp in range(NH):
                            nc.tensor.matmul(
                                pE[:, PGB * h + lb : PGB * h + lb + 1],
                                a_tiles[b][hp][:, h * P : (h + 1) * P],
                                vts[g][:, PGB * hp + lb : PGB * hp + lb + 1],
                                start=(hp == 0),
                                stop=(hp == NH - 1),
                            )
                pEs.append(pE)
            for g in range(PG):
                er = it_pool.tile([P, GCOLS], F32, tag=f"er{g}", name=f"er{g}")
                nc.vector.tensor_mul(er[:], pEs[g][:], rsl(r, g))
                if t < ITER - 1:
                    lg = it_pool.tile([P, GCOLS], F32, tag=f"lg{g}", name=f"lg{g}")
                    nc.vector.tensor_add(lg[:], er[:], rsl(logT_sb, g))
                    vnew = it_pool.tile([P, GCOLS], F32, tag=f"v{g}", name=f"v{g}")
                    nc.scalar.activation(vnew[:], lg[:], AF.Tanh, scale=0.5)
                    vt = it_pool.tile([P, GCOLS], BF16, tag=f"vt{g}", name=f"vt{g}")
                    nc.vector.tensor_mul(vt[:], vnew[:], rsl(r, g))
                    vts[g] = vt
                else:
                    nc.vector.tensor_add(rsl(out_sb, g), er[:], rsl(logT_sb, g))
        nc.sync.dma_start(outT, out_sb[:])

    nc.compile()
    return nc


def _get_nc():
    if "nc" not in _CACHE:
        _CACHE["nc"] = _build_nc()
    return _CACHE["nc"]


# host-side index map: column <-> (batch, half)
_COLMAP = np.empty(COLS, dtype=np.int64)  # col -> b*NH + h
for _b in range(BP):
    for _h in range(NH):
        _COLMAP[_col(_b, _h)] = _b * NH + _h


def _make_in_maps(feats, logits, W):
    wsym = ((W[0] + W[0].T) * 0.5).astype(np.float32)
    in_maps = []
    for i in range(NCORES):
        fs = feats[i * BP : (i + 1) * BP].reshape(BP * N, E)
        ftT = np.ascontiguousarray(fs.T).astype(ml_dtypes.bfloat16)
        lg = logits[i * BP : (i + 1) * BP, :, 0].astype(np.float32)
        # lgT[p, col] = lg[b, h*128+p] for col = _col(b, h)
        lgh = lg.reshape(BP, NH, P)  # [b, h, p]
        lgT = np.empty((P, COLS), dtype=np.float32)
        lgT[:, np.arange(COLS)] = lgh[_COLMAP // NH, _COLMAP % NH, :].T
        in_maps.append({"ftT": ftT, "logT": np.ascontiguousarray(lgT), "wsym": wsym})
    return in_maps


def _unshard(results):
    outs = []
    for i in range(NCORES):
        oT = np.asarray(results[i]["outT"], dtype=np.float32)  # [P, COLS]
        oc = np.empty((BP, NH, P), dtype=np.float32)
        oc[_COLMAP // NH, _COLMAP % NH, :] = oT.T
        outs.append(oc.reshape(BP, N))
    return np.concatenate(outs, axis=0).reshape(B, N, 1).astype(np.float32)


def run(feats, logits, W, trace=False, **kwargs):
    nc = _get_nc()
    in_maps = _make_in_maps(np.asarray(feats), np.asarray(logits), np.asarray(W))
    res = run_bass_kernel_spmd(
        nc, in_maps, core_ids=list(range(NCORES)), trace=trace, **kwargs
    )
    return _unshard(res.results), res


def kernel(feats, logits, W):
    out, _ = run(feats, logits, W)
    return out
